# revision 9
# baseline (speedup 1.0000x reference)
"""Trainium2 Bass kernel for nn_CenterLossN (center-loss style reduction).

Math (per batch n, class c; H=W=384, C=11, N=32):
    res[n,c]   = x[n,c]^2 + centers[n,c]^2 - 2 * x[n,c] @ centers[n,c]
    out[n,h,w] = max_c softmax_c(res)[n,c,h,w] = 1 / sum_c exp(res_c - max_c res_c)
    loss       = sum(clip(out * labels, 1e-12, 1e12)) / (N*H*W)

Device strategy (data-parallel over N across 8 cores, 4 batches/core):
  Host ships, per (n,c) plane: xt2 = (-2*x)^T bf16 (matmul lhsT, so PSUM
  gets -2*x@c directly), cc = centers bf16 (matmul rhs), and
  ee = x^2+centers^2 as fp8e4m3, injected into the same PSUM accumulation
  via an identity-matmul (PSUM += I^T @ ee). PSUM ends up holding
  s = res in fp32 with zero vector-engine work. Per 128-row chunk:
  per-class ACT drain PSUM->bf16, DVE tree-max over 11 classes, per-class
  subtract, one batched ACT exp, DVE tree-add, then label/sum tail.
  clip: only label==0 hits the 1e-12 floor (1/sum >= 1/11 and <= 1);
  host adds 1e-12 * count(labels==0) exactly.
"""

import numpy as np
import ml_dtypes

N, C, H, W = 32, 11, 384, 384
N_CORES = 8
N_LOC = N // N_CORES          # 4 batches per core
PAIRS = N_LOC * C             # 44 (n,c) planes per core
MC = H // 128                 # 3 row-chunks
KC = W // 128                 # 3 contraction chunks

TAIL_MODE = "recip"           # "recip" | "divide" | "gpsimd_divide"
GPSIMD_TREES = False          # offload first tree levels to GpSimd

_BF16 = ml_dtypes.bfloat16
_FP8 = ml_dtypes.float8_e4m3
_COMPILED = None


def _build(n_loc=N_LOC):
    from contextlib import ExitStack
    import concourse.bacc as bacc
    import concourse.tile as tile
    from concourse import mybir

    bf16 = mybir.dt.bfloat16
    f32 = mybir.dt.float32
    fp8 = mybir.dt.float8e4
    AF = mybir.ActivationFunctionType

    nc = bacc.Bacc("TRN2", target_bir_lowering=False, debug=False)

    pairs = n_loc * C
    xt2_d = nc.dram_tensor("xt2", [pairs, W, H], bf16, kind="ExternalInput")
    cc_d = nc.dram_tensor("cc", [pairs, W, H], bf16, kind="ExternalInput")
    ee_d = nc.dram_tensor("ee", [pairs, H, W], fp8, kind="ExternalInput")
    lab_d = nc.dram_tensor("lab", [n_loc, H, W], bf16, kind="ExternalInput")
    id_d = nc.dram_tensor("ident", [128, 128], fp8, kind="ExternalInput")
    out_d = nc.dram_tensor("out", [128, 1], f32, kind="ExternalOutput")

    with ExitStack() as ctx:
        tc = ctx.enter_context(tile.TileContext(nc))
        loads = ctx.enter_context(tc.tile_pool(name="loads", bufs=4))
        spool = ctx.enter_context(tc.tile_pool(name="spool", bufs=3))
        tree = ctx.enter_context(tc.tile_pool(name="tree", bufs=3))
        small = ctx.enter_context(tc.tile_pool(name="small", bufs=4))
        singles = ctx.enter_context(tc.tile_pool(name="singles", bufs=1))
        psum = ctx.enter_context(tc.tile_pool(name="psum", bufs=8, space="PSUM"))

        ident_t = singles.tile([128, 128], fp8)
        nc.sync.dma_start(ident_t[:], id_d[:, :])
        partial = singles.tile([128, n_loc * MC], f32)

        veng = nc.gpsimd if GPSIMD_TREES else nc.vector

        for n in range(n_loc):
            Sn = spool.tile([128, MC, C, W], bf16, tag="S", name=f"S_{n}")
            for c in range(C):
                i = n * C + c
                xt2_t = loads.tile([128, KC, H], bf16, tag="xt2",
                                   name=f"xt2_{n}_{c}")
                nc.sync.dma_start(
                    xt2_t[:], xt2_d[i].rearrange("(kc p) h -> p kc h", p=128)
                )
                cc_t = loads.tile([128, KC, W], bf16, tag="cc",
                                  name=f"cc_{n}_{c}")
                nc.sync.dma_start(
                    cc_t[:], cc_d[i].rearrange("(kc p) w -> p kc w", p=128)
                )
                ee_t = loads.tile([128, MC, W], fp8, tag="ee",
                                  name=f"ee_{n}_{c}")
                nc.sync.dma_start(
                    ee_t[:], ee_d[i].rearrange("(mc p) w -> p mc w", p=128)
                )
                for mc in range(MC):
                    ps = psum.tile([128, W], f32, tag="ps",
                                   name=f"ps_{n}_{c}_{mc}")
                    for kc in range(KC):
                        nc.tensor.matmul(
                            ps[:],
                            xt2_t[:, kc, mc * 128 : (mc + 1) * 128],
                            cc_t[:, kc, :],
                            start=(kc == 0),
                            stop=False,
                        )
                    nc.tensor.matmul(
                        ps[:], ident_t[:], ee_t[:, mc, :], start=False, stop=True
                    )
                    # s (=res) fp32 -> bf16, PSUM -> SBUF on the scalar engine
                    nc.scalar.copy(Sn[:, mc, c, :], ps[:])

            # ---- pass 2, batched across all 3 row-chunks of this n ----
            # running max over classes: 5-way tree
            m5 = tree.tile([128, MC, 5, W], bf16, tag="m5", name=f"m5_{n}")
            veng.tensor_max(m5[:], Sn[:, :, 0:5, :], Sn[:, :, 5:10, :])
            m2 = tree.tile([128, MC, 2, W], bf16, tag="m2", name=f"m2_{n}")
            nc.vector.tensor_max(m2[:], m5[:, :, 0:2, :], m5[:, :, 2:4, :])
            m = small.tile([128, MC, W], bf16, tag="m", name=f"m_{n}")
            nc.vector.tensor_max(m[:], m2[:, :, 0, :], m2[:, :, 1, :])
            nc.vector.tensor_max(m[:], m[:], m5[:, :, 4, :])
            nc.vector.tensor_max(m[:], m[:], Sn[:, :, 10, :])

            # d = s - m, per class (keeps DVE 2x bf16 mode), in place
            for c in range(C):
                nc.vector.tensor_sub(Sn[:, :, c, :], Sn[:, :, c, :], m[:])
            # e = exp(d), in place, one batched op for the whole n
            nc.scalar.activation(Sn[:], Sn[:], AF.Exp)

            # acc = sum_c e : 5-way tree, first levels in place over Sn
            veng.tensor_add(Sn[:, :, 0:5, :], Sn[:, :, 0:5, :], Sn[:, :, 5:10, :])
            nc.vector.tensor_add(Sn[:, :, 0:2, :], Sn[:, :, 0:2, :], Sn[:, :, 2:4, :])
            acc = small.tile([128, MC, W], bf16, tag="acc", name=f"acc_{n}")
            nc.vector.tensor_add(acc[:], Sn[:, :, 0, :], Sn[:, :, 1, :])
            nc.vector.tensor_add(acc[:], acc[:], Sn[:, :, 4, :])
            nc.vector.tensor_add(acc[:], acc[:], Sn[:, :, 10, :])

            labt = loads.tile([128, MC, W], bf16, tag="lab", name=f"lab_{n}")
            nc.sync.dma_start(
                labt[:], lab_d[n].rearrange("(mc p) w -> p mc w", p=128)
            )
            t = small.tile([128, MC, W], f32, tag="t", name=f"t_{n}")
            nc.vector.reciprocal(t[:], acc[:])
            w_t = small.tile([128, MC, W], f32, tag="w", name=f"w_{n}")
            nc.vector.tensor_mul(w_t[:], t[:], labt[:])
            nc.vector.tensor_reduce(
                partial[:, n * MC : (n + 1) * MC],
                w_t[:],
                axis=mybir.AxisListType.X,
                op=mybir.AluOpType.add,
            )

        pf = singles.tile([128, 1], f32)
        nc.vector.tensor_reduce(
            pf[:], partial[:], axis=mybir.AxisListType.X, op=mybir.AluOpType.add
        )
        nc.sync.dma_start(out_d[:, :], pf[:])

    nc.compile()
    return nc


def _get_compiled():
    global _COMPILED
    if _COMPILED is None:
        _COMPILED = _build()
    return _COMPILED


def _host_prep(x, centers, labels):
    x = np.asarray(x, dtype=np.float32)
    centers = np.asarray(centers, dtype=np.float32)
    labels_np = np.asarray(labels)

    n_zero = int((labels_np == 0).sum())

    xt2 = np.ascontiguousarray(
        np.transpose(-2.0 * x, (0, 1, 3, 2))
    ).astype(_BF16)                       # (N, C, W, H)
    cc = centers.astype(_BF16)            # (N, C, H, W)
    ee = (x * x + centers * centers).astype(_FP8)
    lab = labels_np.astype(np.float32).astype(_BF16)  # (N, H, W), values 0..10 exact
    ident = np.eye(128, dtype=_FP8)

    in_maps = []
    for core in range(N_CORES):
        sl = slice(core * N_LOC, (core + 1) * N_LOC)
        in_maps.append(
            {
                "xt2": np.ascontiguousarray(xt2[sl]).reshape(PAIRS, W, H),
                "cc": np.ascontiguousarray(cc[sl]).reshape(PAIRS, H, W),
                "ee": np.ascontiguousarray(ee[sl]).reshape(PAIRS, H, W),
                "lab": np.ascontiguousarray(lab[sl]),
                "ident": ident,
            }
        )
    return in_maps, n_zero


def kernel(x, centers, labels, _trace=False, _trace_kwargs=None):
    from concourse import bass_utils

    nc = _get_compiled()
    in_maps, n_zero = _host_prep(x, centers, labels)

    kwargs = {}
    if _trace:
        kwargs = dict(trace=True, **(_trace_kwargs or {}))
    res = bass_utils.run_bass_kernel_spmd(
        nc, in_maps, core_ids=list(range(N_CORES)), **kwargs
    )

    total = 0.0
    for core in range(N_CORES):
        total += float(res.results[core]["out"].astype(np.float64).sum())
    loss = (total + 1e-12 * n_zero) / float(N * H * W)
    out = np.float32(loss)
    if _trace:
        return out, res
    return out


# revision 11
# speedup vs baseline: 1.1273x; 1.1273x over previous
"""Trainium2 Bass kernel for nn_CenterLossN (center-loss style reduction).

Math (per batch n, class c; H=W=384, C=11, N=32):
    res[n,c]   = x[n,c]^2 + centers[n,c]^2 - 2 * x[n,c] @ centers[n,c]
    out[n,h,w] = max_c softmax_c(res)[n,c,h,w] = 1 / sum_c exp(res_c - max_c res_c)
    loss       = sum(clip(out * labels, 1e-12, 1e12)) / (N*H*W)

Device strategy (data-parallel over N across 8 cores, 4 batches/core):
  Host ships, per (n,c) plane: xt2 = (-2*x)^T bf16 (matmul lhsT, so PSUM
  gets -2*x@c directly), cc = centers bf16 (matmul rhs), and
  ee = x^2+centers^2 as fp8e4m3, injected into the same PSUM accumulation
  via an identity-matmul (PSUM += I^T @ ee). PSUM ends up holding
  s = res in fp32 with zero vector-engine work. Per 128-row chunk:
  per-class ACT drain PSUM->bf16, DVE tree-max over 11 classes, per-class
  subtract, one batched ACT exp, DVE tree-add, then label/sum tail.
  clip: only label==0 hits the 1e-12 floor (1/sum >= 1/11 and <= 1);
  host adds 1e-12 * count(labels==0) exactly.
"""

import numpy as np
import ml_dtypes

N, C, H, W = 32, 11, 384, 384
N_CORES = 8
N_LOC = N // N_CORES          # 4 batches per core
PAIRS = N_LOC * C             # 44 (n,c) planes per core
MC = H // 128                 # 3 row-chunks
KC = W // 128                 # 3 contraction chunks

# notes from HW bring-up on this deployment: AluOpType.divide, Ln activation,
# activation scale!=1.0, tensor_tensor_reduce and custom-DVE ops all fail to
# compile or execute; nc.vector.reciprocal works. GpSimd tensor ops fail to
# compile. Hence the recip tail below and everything on PE/ACT/DVE.
TAIL_MODE = "recip"
GPSIMD_TREES = False

_BF16 = ml_dtypes.bfloat16
_FP8 = ml_dtypes.float8_e4m3
_COMPILED = None


def _build(n_loc=N_LOC):
    from contextlib import ExitStack
    import concourse.bacc as bacc
    import concourse.tile as tile
    from concourse import mybir

    bf16 = mybir.dt.bfloat16
    f32 = mybir.dt.float32
    fp8 = mybir.dt.float8e4
    AF = mybir.ActivationFunctionType

    nc = bacc.Bacc("TRN2", target_bir_lowering=False, debug=False)

    pairs = n_loc * C
    xt2_d = nc.dram_tensor("xt2", [pairs, W, H], bf16, kind="ExternalInput")
    cc_d = nc.dram_tensor("cc", [pairs, W, H], bf16, kind="ExternalInput")
    ee_d = nc.dram_tensor("ee", [pairs, H, W], fp8, kind="ExternalInput")
    lab_d = nc.dram_tensor("lab", [n_loc, H, W], bf16, kind="ExternalInput")
    id_d = nc.dram_tensor("ident", [128, 128], fp8, kind="ExternalInput")
    out_d = nc.dram_tensor("out", [128, 1], f32, kind="ExternalOutput")

    with ExitStack() as ctx:
        tc = ctx.enter_context(tile.TileContext(nc))
        loads = ctx.enter_context(tc.tile_pool(name="loads", bufs=4))
        spool = ctx.enter_context(tc.tile_pool(name="spool", bufs=8))
        tree = ctx.enter_context(tc.tile_pool(name="tree", bufs=6))
        small = ctx.enter_context(tc.tile_pool(name="small", bufs=6))
        singles = ctx.enter_context(tc.tile_pool(name="singles", bufs=1))
        psum = ctx.enter_context(tc.tile_pool(name="psum", bufs=8, space="PSUM"))

        ident_t = singles.tile([128, 128], fp8)
        nc.sync.dma_start(ident_t[:], id_d[:, :])
        partial = singles.tile([128, n_loc * MC], f32)

        veng = nc.gpsimd if GPSIMD_TREES else nc.vector

        for n in range(n_loc):
            s_tiles = [
                spool.tile([128, C, W], bf16, tag="S", name=f"S_{n}_{mc}")
                for mc in range(MC)
            ]
            for c in range(C):
                i = n * C + c
                xt2_t = loads.tile([128, KC, H], bf16, tag="xt2",
                                   name=f"xt2_{n}_{c}")
                nc.sync.dma_start(
                    xt2_t[:], xt2_d[i].rearrange("(kc p) h -> p kc h", p=128)
                )
                cc_t = loads.tile([128, KC, W], bf16, tag="cc",
                                  name=f"cc_{n}_{c}")
                nc.sync.dma_start(
                    cc_t[:], cc_d[i].rearrange("(kc p) w -> p kc w", p=128)
                )
                ee_t = loads.tile([128, MC, W], fp8, tag="ee",
                                  name=f"ee_{n}_{c}")
                nc.sync.dma_start(
                    ee_t[:], ee_d[i].rearrange("(mc p) w -> p mc w", p=128)
                )
                for mc in range(MC):
                    ps = psum.tile([128, W], f32, tag="ps",
                                   name=f"ps_{n}_{c}_{mc}")
                    for kc in range(KC):
                        nc.tensor.matmul(
                            ps[:],
                            xt2_t[:, kc, mc * 128 : (mc + 1) * 128],
                            cc_t[:, kc, :],
                            start=(kc == 0),
                            stop=False,
                        )
                    nc.tensor.matmul(
                        ps[:], ident_t[:], ee_t[:, mc, :], start=False, stop=True
                    )
                    # s (=res) fp32 -> bf16, PSUM -> SBUF on the scalar engine
                    nc.scalar.copy(s_tiles[mc][:, c, :], ps[:])

            for mc in range(MC):
                S = s_tiles[mc]
                # running max over classes: 5-way tree
                m5 = tree.tile([128, 5, W], bf16, tag="m5", name=f"m5_{n}_{mc}")
                veng.tensor_max(m5[:], S[:, 0:5, :], S[:, 5:10, :])
                m2 = tree.tile([128, 2, W], bf16, tag="m2", name=f"m2_{n}_{mc}")
                nc.vector.tensor_max(m2[:], m5[:, 0:2, :], m5[:, 2:4, :])
                m = small.tile([128, W], bf16, tag="m", name=f"m_{n}_{mc}")
                nc.vector.tensor_max(m[:], m2[:, 0, :], m2[:, 1, :])
                nc.vector.tensor_max(m[:], m[:], m5[:, 4, :])
                nc.vector.tensor_max(m[:], m[:], S[:, 10, :])

                # d = s - m, per class (keeps DVE 2x bf16 mode), in place
                for c in range(C):
                    nc.vector.tensor_sub(S[:, c, :], S[:, c, :], m[:])
                # e = exp(d), in place over S, one batched op
                nc.scalar.activation(S[:], S[:], AF.Exp)

                # acc = sum_c e : 5-way tree
                a5 = tree.tile([128, 5, W], bf16, tag="a5", name=f"a5_{n}_{mc}")
                veng.tensor_add(a5[:], S[:, 0:5, :], S[:, 5:10, :])
                a2 = tree.tile([128, 2, W], bf16, tag="a2", name=f"a2_{n}_{mc}")
                nc.vector.tensor_add(a2[:], a5[:, 0:2, :], a5[:, 2:4, :])
                acc = small.tile([128, W], bf16, tag="acc", name=f"acc_{n}_{mc}")
                nc.vector.tensor_add(acc[:], a2[:, 0, :], a2[:, 1, :])
                nc.vector.tensor_add(acc[:], acc[:], a5[:, 4, :])
                nc.vector.tensor_add(acc[:], acc[:], S[:, 10, :])

                labt = loads.tile([128, W], bf16, tag="lab", name=f"lab_{n}_{mc}")
                nc.sync.dma_start(labt[:], lab_d[n, mc * 128 : (mc + 1) * 128, :])
                t = small.tile([128, W], f32, tag="t", name=f"t_{n}_{mc}")
                nc.vector.reciprocal(t[:], acc[:])
                w_t = small.tile([128, W], f32, tag="w", name=f"w_{n}_{mc}")
                nc.vector.tensor_mul(w_t[:], t[:], labt[:])
                slot = n * MC + mc
                nc.vector.tensor_reduce(
                    partial[:, slot : slot + 1],
                    w_t[:],
                    axis=mybir.AxisListType.X,
                    op=mybir.AluOpType.add,
                )

        pf = singles.tile([128, 1], f32)
        nc.vector.tensor_reduce(
            pf[:], partial[:], axis=mybir.AxisListType.X, op=mybir.AluOpType.add
        )
        nc.sync.dma_start(out_d[:, :], pf[:])

    nc.compile()
    return nc


def _get_compiled():
    global _COMPILED
    if _COMPILED is None:
        _COMPILED = _build()
    return _COMPILED


def _host_prep(x, centers, labels):
    x = np.asarray(x, dtype=np.float32)
    centers = np.asarray(centers, dtype=np.float32)
    labels_np = np.asarray(labels)

    n_zero = int((labels_np == 0).sum())

    xt2 = np.ascontiguousarray(
        np.transpose(-2.0 * x, (0, 1, 3, 2))
    ).astype(_BF16)                       # (N, C, W, H)
    cc = centers.astype(_BF16)            # (N, C, H, W)
    ee = (x * x + centers * centers).astype(_FP8)
    lab = labels_np.astype(np.float32).astype(_BF16)  # (N, H, W), values 0..10 exact
    ident = np.eye(128, dtype=_FP8)

    in_maps = []
    for core in range(N_CORES):
        sl = slice(core * N_LOC, (core + 1) * N_LOC)
        in_maps.append(
            {
                "xt2": np.ascontiguousarray(xt2[sl]).reshape(PAIRS, W, H),
                "cc": np.ascontiguousarray(cc[sl]).reshape(PAIRS, H, W),
                "ee": np.ascontiguousarray(ee[sl]).reshape(PAIRS, H, W),
                "lab": np.ascontiguousarray(lab[sl]),
                "ident": ident,
            }
        )
    return in_maps, n_zero


def kernel(x, centers, labels, _trace=False, _trace_kwargs=None):
    from concourse import bass_utils

    nc = _get_compiled()
    in_maps, n_zero = _host_prep(x, centers, labels)

    kwargs = {}
    if _trace:
        kwargs = dict(trace=True, **(_trace_kwargs or {}))
    res = bass_utils.run_bass_kernel_spmd(
        nc, in_maps, core_ids=list(range(N_CORES)), **kwargs
    )

    total = 0.0
    for core in range(N_CORES):
        total += float(res.results[core]["out"].astype(np.float64).sum())
    loss = (total + 1e-12 * n_zero) / float(N * H * W)
    out = np.float32(loss)
    if _trace:
        return out, res
    return out


# revision 13
# speedup vs baseline: 1.1854x; 1.0516x over previous
"""Trainium2 Bass kernel for nn_CenterLossN (center-loss style reduction).

Math (per batch n, class c; H=W=384, C=11, N=32):
    res[n,c]   = x[n,c]^2 + centers[n,c]^2 - 2 * x[n,c] @ centers[n,c]
    out[n,h,w] = max_c softmax_c(res)[n,c,h,w] = 1 / sum_c exp(res_c - max_c res_c)
    loss       = sum(clip(out * labels, 1e-12, 1e12)) / (N*H*W)

Device strategy (data-parallel over N across 8 cores, 4 batches/core):
  Host ships, per (n,c) plane: xt2 = (-2*x)^T bf16 (matmul lhsT, so PSUM
  gets -2*x@c directly), cc = centers bf16 (matmul rhs), and
  ee = x^2+centers^2 as fp8e4m3, injected into the same PSUM accumulation
  via an identity-matmul (PSUM += I^T @ ee). PSUM ends up holding
  s = res in fp32 with zero vector-engine work. Per 128-row chunk:
  per-class ACT drain PSUM->bf16, DVE tree-max over 11 classes, per-class
  subtract, one batched ACT exp, DVE tree-add, then label/sum tail.
  clip: only label==0 hits the 1e-12 floor (1/sum >= 1/11 and <= 1);
  host adds 1e-12 * count(labels==0) exactly.
"""

import numpy as np
import ml_dtypes

N, C, H, W = 32, 11, 384, 384
N_CORES = 8
N_LOC = N // N_CORES          # 4 batches per core
PAIRS = N_LOC * C             # 44 (n,c) planes per core
MC = H // 128                 # 3 row-chunks
KC = W // 128                 # 3 contraction chunks

# notes from HW bring-up on this deployment: AluOpType.divide, Ln activation,
# activation scale!=1.0, tensor_tensor_reduce and custom-DVE ops all fail to
# compile or execute; nc.vector.reciprocal works. GpSimd tensor ops fail to
# compile. Hence the recip tail below and everything on PE/ACT/DVE.
TAIL_MODE = "recip"
GPSIMD_TREES = False

_BF16 = ml_dtypes.bfloat16
_FP8 = ml_dtypes.float8_e4m3
_COMPILED = None


def _build(n_loc=N_LOC):
    from contextlib import ExitStack
    import concourse.bacc as bacc
    import concourse.tile as tile
    from concourse import mybir

    bf16 = mybir.dt.bfloat16
    f32 = mybir.dt.float32
    fp8 = mybir.dt.float8e4
    AF = mybir.ActivationFunctionType

    nc = bacc.Bacc("TRN2", target_bir_lowering=False, debug=False)

    pairs = n_loc * C
    xt2_d = nc.dram_tensor("xt2", [pairs, W, H], bf16, kind="ExternalInput")
    cc_d = nc.dram_tensor("cc", [pairs, W, H], bf16, kind="ExternalInput")
    ee_d = nc.dram_tensor("ee", [pairs, H, W], fp8, kind="ExternalInput")
    lab_d = nc.dram_tensor("lab", [n_loc, H, W], bf16, kind="ExternalInput")
    id_d = nc.dram_tensor("ident", [128, 128], fp8, kind="ExternalInput")
    out_d = nc.dram_tensor("out", [128, 1], f32, kind="ExternalOutput")

    with ExitStack() as ctx:
        tc = ctx.enter_context(tile.TileContext(nc))
        loads = ctx.enter_context(tc.tile_pool(name="loads", bufs=6))
        dpool = ctx.enter_context(tc.tile_pool(name="dpool", bufs=3))
        spool = ctx.enter_context(tc.tile_pool(name="spool", bufs=6))
        tree = ctx.enter_context(tc.tile_pool(name="tree", bufs=4))
        small = ctx.enter_context(tc.tile_pool(name="small", bufs=6))
        singles = ctx.enter_context(tc.tile_pool(name="singles", bufs=1))
        psum = ctx.enter_context(tc.tile_pool(name="psum", bufs=8, space="PSUM"))

        ident_t = singles.tile([128, 128], fp8)
        nc.sync.dma_start(ident_t[:], id_d[:, :])
        partial = singles.tile([128, n_loc * MC], f32)

        veng = nc.gpsimd if GPSIMD_TREES else nc.vector

        for n in range(n_loc):
            s_tiles = [
                spool.tile([128, C, W], bf16, tag="S", name=f"S_{n}_{mc}")
                for mc in range(MC)
            ]
            for c in range(C):
                i = n * C + c
                xt2_t = loads.tile([128, KC, H], bf16, tag="xt2",
                                   name=f"xt2_{n}_{c}")
                nc.sync.dma_start(
                    xt2_t[:], xt2_d[i].rearrange("(kc p) h -> p kc h", p=128)
                )
                cc_t = loads.tile([128, KC, W], bf16, tag="cc",
                                  name=f"cc_{n}_{c}")
                nc.sync.dma_start(
                    cc_t[:], cc_d[i].rearrange("(kc p) w -> p kc w", p=128)
                )
                ee_t = loads.tile([128, MC, W], fp8, tag="ee",
                                  name=f"ee_{n}_{c}")
                nc.gpsimd.dma_start(
                    ee_t[:], ee_d[i].rearrange("(mc p) w -> p mc w", p=128)
                )
                for mc in range(MC):
                    ps = psum.tile([128, W], f32, tag="ps",
                                   name=f"ps_{n}_{c}_{mc}")
                    for kc in range(KC):
                        nc.tensor.matmul(
                            ps[:],
                            xt2_t[:, kc, mc * 128 : (mc + 1) * 128],
                            cc_t[:, kc, :],
                            start=(kc == 0),
                            stop=False,
                        )
                    nc.tensor.matmul(
                        ps[:], ident_t[:], ee_t[:, mc, :], start=False, stop=True
                    )
                    # s (=res) fp32 -> bf16, PSUM -> SBUF on the scalar engine
                    nc.scalar.copy(s_tiles[mc][:, c, :], ps[:])

            for mc in range(MC):
                S = s_tiles[mc]
                # running max over classes: 5-way tree
                m5 = tree.tile([128, 5, W], bf16, tag="m5", name=f"m5_{n}_{mc}")
                veng.tensor_max(m5[:], S[:, 0:5, :], S[:, 5:10, :])
                m2 = tree.tile([128, 2, W], bf16, tag="m2", name=f"m2_{n}_{mc}")
                nc.vector.tensor_max(m2[:], m5[:, 0:2, :], m5[:, 2:4, :])
                m = small.tile([128, W], bf16, tag="m", name=f"m_{n}_{mc}")
                nc.vector.tensor_max(m[:], m2[:, 0, :], m2[:, 1, :])
                nc.vector.tensor_max(m[:], m[:], m5[:, 4, :])
                nc.vector.tensor_max(m[:], m[:], S[:, 10, :])

                # d = s - m, per class, into a fresh tile (in-place TT
                # was measured slower; fresh dst keeps the 2x bf16 uop)
                D = dpool.tile([128, C, W], bf16, tag="D", name=f"D_{n}_{mc}")
                for c in range(C):
                    nc.vector.tensor_sub(D[:, c, :], S[:, c, :], m[:])
                # e = exp(d), one batched op (ACT rate is mode-independent)
                nc.scalar.activation(D[:], D[:], AF.Exp)

                # acc = sum_c e : 5-way tree
                a5 = tree.tile([128, 5, W], bf16, tag="a5", name=f"a5_{n}_{mc}")
                veng.tensor_add(a5[:], D[:, 0:5, :], D[:, 5:10, :])
                a2 = tree.tile([128, 2, W], bf16, tag="a2", name=f"a2_{n}_{mc}")
                nc.vector.tensor_add(a2[:], a5[:, 0:2, :], a5[:, 2:4, :])
                acc = small.tile([128, W], bf16, tag="acc", name=f"acc_{n}_{mc}")
                nc.vector.tensor_add(acc[:], a2[:, 0, :], a2[:, 1, :])
                nc.vector.tensor_add(acc[:], acc[:], a5[:, 4, :])
                nc.vector.tensor_add(acc[:], acc[:], D[:, 10, :])

                labt = loads.tile([128, W], bf16, tag="lab", name=f"lab_{n}_{mc}")
                nc.gpsimd.dma_start(labt[:], lab_d[n, mc * 128 : (mc + 1) * 128, :])
                t = small.tile([128, W], f32, tag="t", name=f"t_{n}_{mc}")
                nc.vector.reciprocal(t[:], acc[:])
                w_t = small.tile([128, W], f32, tag="w", name=f"w_{n}_{mc}")
                slot = n * MC + mc
                nc.vector.scalar_tensor_tensor(
                    out=w_t[:], in0=labt[:], scalar=0.0, in1=t[:],
                    op0=mybir.AluOpType.add, op1=mybir.AluOpType.mult,
                    accum_out=partial[:, slot : slot + 1],
                )

        pf = singles.tile([128, 1], f32)
        nc.vector.tensor_reduce(
            pf[:], partial[:], axis=mybir.AxisListType.X, op=mybir.AluOpType.add
        )
        nc.sync.dma_start(out_d[:, :], pf[:])

    nc.compile()
    return nc


def _get_compiled():
    global _COMPILED
    if _COMPILED is None:
        _COMPILED = _build()
    return _COMPILED


def _host_prep(x, centers, labels):
    x = np.asarray(x, dtype=np.float32)
    centers = np.asarray(centers, dtype=np.float32)
    labels_np = np.asarray(labels)

    n_zero = int((labels_np == 0).sum())

    xt2 = np.ascontiguousarray(
        np.transpose(-2.0 * x, (0, 1, 3, 2))
    ).astype(_BF16)                       # (N, C, W, H)
    cc = centers.astype(_BF16)            # (N, C, H, W)
    ee = (x * x + centers * centers).astype(_FP8)
    lab = labels_np.astype(np.float32).astype(_BF16)  # (N, H, W), values 0..10 exact
    ident = np.eye(128, dtype=_FP8)

    in_maps = []
    for core in range(N_CORES):
        sl = slice(core * N_LOC, (core + 1) * N_LOC)
        in_maps.append(
            {
                "xt2": np.ascontiguousarray(xt2[sl]).reshape(PAIRS, W, H),
                "cc": np.ascontiguousarray(cc[sl]).reshape(PAIRS, H, W),
                "ee": np.ascontiguousarray(ee[sl]).reshape(PAIRS, H, W),
                "lab": np.ascontiguousarray(lab[sl]),
                "ident": ident,
            }
        )
    return in_maps, n_zero


def kernel(x, centers, labels, _trace=False, _trace_kwargs=None):
    from concourse import bass_utils

    nc = _get_compiled()
    in_maps, n_zero = _host_prep(x, centers, labels)

    kwargs = {}
    if _trace:
        kwargs = dict(trace=True, **(_trace_kwargs or {}))
    res = bass_utils.run_bass_kernel_spmd(
        nc, in_maps, core_ids=list(range(N_CORES)), **kwargs
    )

    total = 0.0
    for core in range(N_CORES):
        total += float(res.results[core]["out"].astype(np.float64).sum())
    loss = (total + 1e-12 * n_zero) / float(N * H * W)
    out = np.float32(loss)
    if _trace:
        return out, res
    return out


# revision 14
# speedup vs baseline: 1.2526x; 1.0567x over previous
"""Trainium2 Bass kernel for nn_CenterLossN (center-loss style reduction).

Math (per batch n, class c; H=W=384, C=11, N=32):
    res[n,c]   = x[n,c]^2 + centers[n,c]^2 - 2 * x[n,c] @ centers[n,c]
    out[n,h,w] = max_c softmax_c(res)[n,c,h,w] = 1 / sum_c exp(res_c - max_c res_c)
    loss       = sum(clip(out * labels, 1e-12, 1e12)) / (N*H*W)

Device strategy (data-parallel over N across 8 cores, 4 batches/core):
  Host ships, per (n,c) plane: xt2 = (-2*x)^T bf16 (matmul lhsT, so PSUM
  gets -2*x@c directly), cc = centers bf16 (matmul rhs), and
  ee = x^2+centers^2 as fp8e4m3, injected into the same PSUM accumulation
  via an identity-matmul (PSUM += I^T @ ee). PSUM ends up holding
  s = res in fp32 with zero vector-engine work. Per 128-row chunk:
  per-class ACT drain PSUM->bf16, DVE tree-max over 11 classes, per-class
  subtract, one batched ACT exp, DVE tree-add, then label/sum tail.
  clip: only label==0 hits the 1e-12 floor (1/sum >= 1/11 and <= 1);
  host adds 1e-12 * count(labels==0) exactly.
"""

import numpy as np
import ml_dtypes

N, C, H, W = 32, 11, 384, 384
N_CORES = 8
N_LOC = N // N_CORES          # 4 batches per core
PAIRS = N_LOC * C             # 44 (n,c) planes per core
MC = H // 128                 # 3 row-chunks
KC = W // 128                 # 3 contraction chunks

# notes from HW bring-up on this deployment: AluOpType.divide, Ln activation,
# activation scale!=1.0, tensor_tensor_reduce and custom-DVE ops all fail to
# compile or execute; nc.vector.reciprocal works. GpSimd tensor ops fail to
# compile. Hence the recip tail below and everything on PE/ACT/DVE.
TAIL_MODE = "recip"
GPSIMD_TREES = False

_BF16 = ml_dtypes.bfloat16
_FP8 = ml_dtypes.float8_e4m3
_COMPILED = None


def _build(n_loc=N_LOC):
    from contextlib import ExitStack
    import concourse.bass as bass
    import concourse.bacc as bacc
    import concourse.tile as tile
    from concourse import mybir

    bf16 = mybir.dt.bfloat16
    f32 = mybir.dt.float32
    fp8 = mybir.dt.float8e4
    AF = mybir.ActivationFunctionType

    nc = bacc.Bacc("TRN2", target_bir_lowering=False, debug=False)

    pairs = n_loc * C
    xt2_d = nc.dram_tensor("xt2", [pairs, W, H], bf16, kind="ExternalInput")
    cc_d = nc.dram_tensor("cc", [pairs, W, H], bf16, kind="ExternalInput")
    ee_d = nc.dram_tensor("ee", [pairs, H, W], fp8, kind="ExternalInput")
    lab_d = nc.dram_tensor("lab", [n_loc, H, W], bf16, kind="ExternalInput")
    id_d = nc.dram_tensor("ident", [128, 128], fp8, kind="ExternalInput")
    out_d = nc.dram_tensor("out", [128, 1], f32, kind="ExternalOutput")

    with ExitStack() as ctx:
        tc = ctx.enter_context(tile.TileContext(nc))
        loads = ctx.enter_context(tc.tile_pool(name="loads", bufs=6))
        dpool = ctx.enter_context(tc.tile_pool(name="dpool", bufs=3))
        spool = ctx.enter_context(tc.tile_pool(name="spool", bufs=6))
        tree = ctx.enter_context(tc.tile_pool(name="tree", bufs=4))
        small = ctx.enter_context(tc.tile_pool(name="small", bufs=6))
        singles = ctx.enter_context(tc.tile_pool(name="singles", bufs=1))
        psum = ctx.enter_context(tc.tile_pool(name="psum", bufs=8, space="PSUM"))

        ident_t = singles.tile([128, 128], fp8)
        nc.sync.dma_start(ident_t[:], id_d[:, :])
        partial = singles.tile([128, n_loc * MC], f32)

        veng = nc.gpsimd if GPSIMD_TREES else nc.vector

        for n in range(n_loc):
            s_tiles = [
                spool.tile([128, C, W], bf16, tag="S", name=f"S_{n}_{mc}")
                for mc in range(MC)
            ]
            for c in range(C):
                i = n * C + c
                xt2_t = loads.tile([128, KC, H], bf16, tag="xt2",
                                   name=f"xt2_{n}_{c}")
                nc.sync.dma_start(
                    xt2_t[:], xt2_d[i].rearrange("(kc p) h -> p kc h", p=128)
                )
                cc_t = loads.tile([128, KC, W], bf16, tag="cc",
                                  name=f"cc_{n}_{c}")
                nc.sync.dma_start(
                    cc_t[:], cc_d[i].rearrange("(kc p) w -> p kc w", p=128)
                )
                ee_t = loads.tile([128, MC, W], fp8, tag="ee",
                                  name=f"ee_{n}_{c}")
                nc.gpsimd.dma_start(
                    ee_t[:], ee_d[i].rearrange("(mc p) w -> p mc w", p=128)
                )
                for mc in range(MC):
                    ps = psum.tile([128, W], f32, tag="ps",
                                   name=f"ps_{n}_{c}_{mc}")
                    for kc in range(KC):
                        nc.tensor.matmul(
                            ps[:],
                            xt2_t[:, kc, mc * 128 : (mc + 1) * 128],
                            cc_t[:, kc, :],
                            start=(kc == 0),
                            stop=False,
                        )
                    nc.tensor.matmul(
                        ps[:], ident_t[:], ee_t[:, mc, :], start=False, stop=True
                    )
                    # s (=res) fp32 -> bf16, PSUM -> SBUF on the scalar engine
                    nc.scalar.copy(s_tiles[mc][:, c, :], ps[:])

            for mc in range(MC):
                S = s_tiles[mc]
                # running max over classes: 5-way tree
                m5 = tree.tile([128, 5, W], bf16, tag="m5", name=f"m5_{n}_{mc}")
                veng.tensor_max(m5[:], S[:, 0:5, :], S[:, 5:10, :])
                m2 = tree.tile([128, 2, W], bf16, tag="m2", name=f"m2_{n}_{mc}")
                nc.vector.tensor_max(m2[:], m5[:, 0:2, :], m5[:, 2:4, :])
                m = small.tile([128, W], bf16, tag="m", name=f"m_{n}_{mc}")
                nc.vector.tensor_max(m[:], m2[:, 0, :], m2[:, 1, :])
                nc.vector.tensor_max(m[:], m[:], m5[:, 4, :])
                nc.vector.tensor_max(m[:], m[:], S[:, 10, :])

                # d = s - m in ONE op: m broadcast along the class dim via a
                # step-0 AP (runs at 1x but beats 11 separate 2x ops + overhead)
                D = dpool.tile([128, C, W], bf16, tag="D", name=f"D_{n}_{mc}")
                m_ap = m[:]
                m_b = bass.AP(
                    tensor=m_ap.tensor, offset=m_ap.offset,
                    ap=[list(m_ap.ap[0]), [0, C], list(m_ap.ap[1])],
                )
                nc.vector.tensor_sub(D[:], S[:], m_b)
                # e = exp(d), one batched op (ACT rate is mode-independent)
                nc.scalar.activation(D[:], D[:], AF.Exp)

                # acc = sum_c e : 5-way tree
                a5 = tree.tile([128, 5, W], bf16, tag="a5", name=f"a5_{n}_{mc}")
                veng.tensor_add(a5[:], D[:, 0:5, :], D[:, 5:10, :])
                a2 = tree.tile([128, 2, W], bf16, tag="a2", name=f"a2_{n}_{mc}")
                nc.vector.tensor_add(a2[:], a5[:, 0:2, :], a5[:, 2:4, :])
                acc = small.tile([128, W], bf16, tag="acc", name=f"acc_{n}_{mc}")
                nc.vector.tensor_add(acc[:], a2[:, 0, :], a2[:, 1, :])
                nc.vector.tensor_add(acc[:], acc[:], a5[:, 4, :])
                nc.vector.tensor_add(acc[:], acc[:], D[:, 10, :])

                labt = loads.tile([128, W], bf16, tag="lab", name=f"lab_{n}_{mc}")
                nc.gpsimd.dma_start(labt[:], lab_d[n, mc * 128 : (mc + 1) * 128, :])
                t = small.tile([128, W], f32, tag="t", name=f"t_{n}_{mc}")
                nc.vector.reciprocal(t[:], acc[:])
                w_t = small.tile([128, W], f32, tag="w", name=f"w_{n}_{mc}")
                slot = n * MC + mc
                nc.vector.scalar_tensor_tensor(
                    out=w_t[:], in0=labt[:], scalar=0.0, in1=t[:],
                    op0=mybir.AluOpType.add, op1=mybir.AluOpType.mult,
                    accum_out=partial[:, slot : slot + 1],
                )

        pf = singles.tile([128, 1], f32)
        nc.vector.tensor_reduce(
            pf[:], partial[:], axis=mybir.AxisListType.X, op=mybir.AluOpType.add
        )
        nc.sync.dma_start(out_d[:, :], pf[:])

    nc.compile()
    return nc


def _get_compiled():
    global _COMPILED
    if _COMPILED is None:
        _COMPILED = _build()
    return _COMPILED


def _host_prep(x, centers, labels):
    x = np.asarray(x, dtype=np.float32)
    centers = np.asarray(centers, dtype=np.float32)
    labels_np = np.asarray(labels)

    n_zero = int((labels_np == 0).sum())

    xt2 = np.ascontiguousarray(
        np.transpose(-2.0 * x, (0, 1, 3, 2))
    ).astype(_BF16)                       # (N, C, W, H)
    cc = centers.astype(_BF16)            # (N, C, H, W)
    ee = (x * x + centers * centers).astype(_FP8)
    lab = labels_np.astype(np.float32).astype(_BF16)  # (N, H, W), values 0..10 exact
    ident = np.eye(128, dtype=_FP8)

    in_maps = []
    for core in range(N_CORES):
        sl = slice(core * N_LOC, (core + 1) * N_LOC)
        in_maps.append(
            {
                "xt2": np.ascontiguousarray(xt2[sl]).reshape(PAIRS, W, H),
                "cc": np.ascontiguousarray(cc[sl]).reshape(PAIRS, H, W),
                "ee": np.ascontiguousarray(ee[sl]).reshape(PAIRS, H, W),
                "lab": np.ascontiguousarray(lab[sl]),
                "ident": ident,
            }
        )
    return in_maps, n_zero


def kernel(x, centers, labels, _trace=False, _trace_kwargs=None):
    from concourse import bass_utils

    nc = _get_compiled()
    in_maps, n_zero = _host_prep(x, centers, labels)

    kwargs = {}
    if _trace:
        kwargs = dict(trace=True, **(_trace_kwargs or {}))
    res = bass_utils.run_bass_kernel_spmd(
        nc, in_maps, core_ids=list(range(N_CORES)), **kwargs
    )

    total = 0.0
    for core in range(N_CORES):
        total += float(res.results[core]["out"].astype(np.float64).sum())
    loss = (total + 1e-12 * n_zero) / float(N * H * W)
    out = np.float32(loss)
    if _trace:
        return out, res
    return out
